# revision 8
# baseline (speedup 1.0000x reference)
"""Trainium2 Bass kernel: batched single-query cross-attention (gated-V variant).

Reference computation (per batch b):
    Q  = query @ Wq + bq                     (1, E)
    K  = key @ Wk + bk                       (S, E)
    V  = value @ Wv + bv                     (S, E)
    scores[h, s] = <Q_head_h, K_head_h[s]> / sqrt(D)  (+ mask)
    aw = softmax_s(scores)                   (H, S)
    attn[s, e] = V[s, e] * aw[e // D, s]
    out = attn @ Wo + bo                     (S, E)
    returns (out, broadcast(aw to (B, H, S, D)))

Key algebraic facts exploited:
  * K is only consumed through scores, and scores = key @ A with
    A[e, h] = (1/8) * sum_d Wk[e, h*64+d] * Q[h*64+d]  -- the full K
    projection (8.6 GFLOP/core) collapses into a 1024x16 matrix A built
    with cheap elementwise ops. bk shifts every score of a head by the
    same constant, which softmax cancels, so bk is dropped entirely.
  * Everything is kept in "embedding-on-partitions" layout (x^T tiles) so
    all big matmuls use naturally-laid-out weights; key/value are
    transposed on the host (pure layout prep, zero FLOPs).

Sharding: data-parallel over the batch dim, one batch element per core
(B == 8 == n_cores). Weights are replicated.

Precision: the score/softmax chain runs in float32r (TF32-like PE mode,
~1.5e-4 matmul error) so the attention-weight output stays accurate; the
two heavy projections (V and output, 2 x 8.6 GFLOP/core) run in bf16,
which streams at twice the fp32r rate on the PE, accumulating in fp32.

Scheduling notes: a burst of K=1 filler matmuls is woven into the
DMA-bound warmup so the PE's HAM clock-gate reaches 2.4 GHz before the
score matmuls, and stays there through the keyT-streaming phase (idle
gaps stay under the ~3.4 us re-throttle window).
"""

import sys

import numpy as np

if "/opt/trn_rl_repo" not in sys.path:
    sys.path.insert(0, "/opt/trn_rl_repo")

import ml_dtypes  # noqa: E402

import concourse.bass as bass  # noqa: E402  # noqa: F401
import concourse.tile as tile  # noqa: E402
from concourse import bacc, mybir  # noqa: E402
from concourse.bass_utils import run_bass_kernel_spmd  # noqa: E402

B, S, E, H, D = 8, 4096, 1024, 16, 64
NT = E // 128  # 8 partition-tiles of the embedding dim
NCH = S // 512  # 8 sequence chunks
N_CORES = 8
F32 = mybir.dt.float32
FR = mybir.dt.float32r
BF = mybir.dt.bfloat16
BF_NP = ml_dtypes.bfloat16

LAST_RESULT = None
_compiled_nc = None


def _build():
    nc = bacc.Bacc(
        "TRN2", target_bir_lowering=False, debug=False, num_devices=N_CORES
    )

    def din(name, shape, dt=F32):
        return nc.dram_tensor(name, shape, dt, kind="ExternalInput").ap()

    def dout(name, shape):
        return nc.dram_tensor(name, shape, F32, kind="ExternalOutput").ap()

    # score-path tensors are float32r end to end (the BIR verifier requires
    # fp32r matmul operands to be produced as fp32r); V/O-path tensors bf16.
    q_col = din("q_col", (128, NT), FR)  # query[b] as column tiles: [p, k] = q[k*128+p]
    keyT = din("keyT", (E, S), FR)       # key[b].T
    valueT = din("valueT", (E, S), BF)   # value[b].T
    mb = din("maskbias", (1, S), FR)     # (mask-1)*1e9 : 0 keep, -1e9 masked
    Wq = din("Wq", (E, E), FR)
    Wk = din("Wk", (E, E))               # only feeds the DVE (A build)
    Wv = din("Wv", (E, E), BF)
    Wo = din("Wo", (E, E), BF)
    bq = din("bq", (1, E))
    bv_col = din("bv_col", (128, NT))    # bv as column tiles
    bo = din("bo", (1, E))
    ones = din("ones", (1, E), FR)
    expm = din("EXP", (H, E), BF)        # EXP[h, e] = 1.0 if e // 64 == h else 0
    out = dout("out", (S, E))
    aw = dout("aw", (H, S))

    AluOp = mybir.AluOpType
    Act = mybir.ActivationFunctionType

    with tile.TileContext(nc) as tc:
        with (
            tc.tile_pool(name="constp", bufs=1) as constp,
            tc.tile_pool(name="wres", bufs=1) as wres,
            tc.tile_pool(name="wstr", bufs=2) as wstr,
            tc.tile_pool(name="kstr", bufs=10) as kstr,
            tc.tile_pool(name="vstr", bufs=16) as vstr,
            tc.tile_pool(name="smax", bufs=1) as smax,
            tc.tile_pool(name="awbp", bufs=3) as awbp,
            tc.tile_pool(name="attnp", bufs=12) as attnp,
            tc.tile_pool(name="outp", bufs=4) as outp,
            tc.tile_pool(name="ps", bufs=2, space="PSUM") as ps,
        ):
            # ---- small constants; exp first so PE warmup fillers can start ----
            exp_sb = constp.tile([H, E], BF, tag="exp")
            nc.sync.dma_start(exp_sb[:], expm[:])
            q_sb = constp.tile([128, NT], FR, tag="q_col")
            nc.sync.dma_start(q_sb[:], q_col[:])
            bq_sb = constp.tile([1, E], F32, tag="bq")
            nc.sync.dma_start(bq_sb[:], bq[:])
            bvc_sb = constp.tile([128, NT], F32, tag="bvc")
            nc.sync.dma_start(bvc_sb[:], bv_col[:])
            bo_sb = constp.tile([1, E], F32, tag="bo")
            nc.sync.dma_start(bo_sb[:], bo[:])
            ones_sb = constp.tile([1, E], FR, tag="ones")
            nc.sync.dma_start(ones_sb[:], ones[:])
            mb_sb = constp.tile([1, S], FR, tag="mb")
            nc.sync.dma_start(mb_sb[:], mb[:])

            # HAM warmup: dead K=1 matmuls into a scratch PSUM bank keep the
            # PE busy through the DMA-bound startup so it upclocks early.
            filler_n = iter(range(1000))

            def filler(count):
                for _ in range(count):
                    f_ps = ps.tile(
                        [128, 512], F32, tag="ps_s", name=f"fill{next(filler_n)}"
                    )
                    nc.tensor.matmul(
                        f_ps[:],
                        exp_sb[:1, :128],
                        exp_sb[:1, :512],
                        start=True,
                        stop=True,
                    )

            filler(16)

            # ---- Q projection: Q = q @ Wq + bq, as a [1, E] row ----
            qrow_ps = [ps.tile([1, 512], F32, tag="ps_v", name=f"qrow{c}") for c in range(2)]
            for k in range(NT):
                wq_k = wstr.tile([128, E], FR, tag="w", name=f"wq{k}")
                nc.sync.dma_start(wq_k[:], Wq[k * 128 : (k + 1) * 128, :])
                for c in range(2):
                    nc.tensor.matmul(
                        qrow_ps[c][:],
                        q_sb[:, k : k + 1],
                        wq_k[:, c * 512 : (c + 1) * 512],
                        start=(k == 0),
                        stop=(k == NT - 1),
                    )
                filler(4)
            q_row = constp.tile([1, E], FR, tag="q_row")
            for c in range(2):
                nc.vector.tensor_add(
                    q_row[:, c * 512 : (c + 1) * 512],
                    qrow_ps[c][:],
                    bq_sb[:, c * 512 : (c + 1) * 512],
                )
            # fold in the 1/sqrt(D) score scale here (scores are linear in Q)
            nc.scalar.mul(q_row[:], q_row[:], 1.0 / 8.0)

            # ---- broadcast Q across partitions (PE rank-1 trick) ----
            qb_sb = constp.tile([128, E], F32, tag="qb")
            for c in range(2):
                qb_ps = ps.tile([128, 512], F32, tag="ps_v", name=f"qb{c}")
                nc.tensor.matmul(
                    qb_ps[:],
                    ones_sb[:1, :128],
                    q_row[:1, c * 512 : (c + 1) * 512],
                    start=True,
                    stop=True,
                )
                nc.vector.tensor_copy(qb_sb[:, c * 512 : (c + 1) * 512], qb_ps[:])

            # ---- A[e, h] = sum over the 64-wide head block of Wk * Qb;
            # keyT chunk-0 tiles prefetch behind each Wk tile ----
            a_sb = constp.tile([128, 128], FR, tag="A")
            kt0 = []
            for k in range(NT):
                wk_k = wstr.tile([128, E], F32, tag="wk", name=f"wk{k}", bufs=2)
                nc.sync.dma_start(wk_k[:], Wk[k * 128 : (k + 1) * 128, :])
                kt = kstr.tile([128, 512], FR, tag="kt", name=f"kt0_{k}")
                nc.sync.dma_start(kt[:], keyT[k * 128 : (k + 1) * 128, 0:512])
                kt0.append(kt)
                tmp = wstr.tile([128, E], F32, tag="w", name=f"tmpA{k}")
                nc.vector.tensor_mul(tmp[:], wk_k[:], qb_sb[:])
                with nc.allow_low_precision(reason="fp32r rounding of A"):
                    nc.vector.reduce_sum(
                        a_sb[:, k * H : (k + 1) * H],
                        tmp[:].rearrange("p (h d) -> p h d", d=D),
                        axis=mybir.AxisListType.X,
                    )
                filler(4)

            # ---- scores^T[h, s] = sum_e A[e, h] * keyT[e, s]  (+ mask) ----
            sc_sb = smax.tile([H, S], FR, tag="sc")
            aw_bf = smax.tile([H, S], BF, tag="awbf")
            shift_sb = constp.tile([H, 1], F32, tag="shift")
            nc.vector.memset(shift_sb[:], -12.0)
            for n in range(NCH):
                cs = slice(n * 512, (n + 1) * 512)
                s_ps = ps.tile([H, 512], F32, tag="ps_s", name=f"sps{n}")
                for t in range(NT):
                    if n == 0:
                        kt = kt0[t]
                    else:
                        kt = kstr.tile([128, 512], FR, tag="kt", name=f"kt{n}_{t}")
                        nc.sync.dma_start(kt[:], keyT[t * 128 : (t + 1) * 128, cs])
                    nc.tensor.matmul(
                        s_ps[:],
                        a_sb[:, t * H : (t + 1) * H],
                        kt[:],
                        start=(t == 0),
                        stop=False,
                    )
                nc.tensor.matmul(
                    s_ps[:],
                    ones_sb[:1, :H],
                    mb_sb[:1, cs],
                    start=False,
                    stop=True,
                )
                # online softmax: exp with a fixed safe shift (scores are
                # O(1); true max <= ~8 << 12), normalization folded in later
                nc.scalar.activation(sc_sb[:, cs], s_ps[:], Act.Exp, bias=shift_sb[:])
                with nc.allow_low_precision(reason="bf16 gate weights"):
                    nc.vector.tensor_copy(aw_bf[:, cs], sc_sb[:, cs])
                filler(6)

            # ---- V/O weights + first value chunk; queued after the
            # latency-critical score-path loads, before they are needed ----
            wv_sb = wres.tile([128, NT * E], BF, tag="wv")
            wo_sb = wres.tile([128, NT * E], BF, tag="wo")
            for k in range(NT):
                nc.sync.dma_start(
                    wv_sb[:, k * E : (k + 1) * E], Wv[k * 128 : (k + 1) * 128, :]
                )
            vt0 = []
            for t in range(NT):
                vt = vstr.tile([128, 512], BF, tag="vt", name=f"vt0_{t}")
                nc.sync.dma_start(vt[:], valueT[t * 128 : (t + 1) * 128, 0:512])
                vt0.append(vt)
            for k in range(NT):
                nc.sync.dma_start(
                    wo_sb[:, k * E : (k + 1) * E], Wo[k * 128 : (k + 1) * 128, :]
                )
            # bo broadcast across partitions, exact, on the otherwise-idle GpSimd
            bob_sb = constp.tile([128, E], F32, tag="bob")
            nc.gpsimd.partition_broadcast(bob_sb[:], bo_sb[:1, :])

            # ---- softmax tail: global sum, reciprocal, and the per-row
            # expansion of 1/sum to the awB partition layout ----
            sumexp = constp.tile([H, 1], F32, tag="sumexp")
            nc.vector.reduce_sum(sumexp[:], sc_sb[:], axis=mybir.AxisListType.X)
            rinv = constp.tile([H, 1], F32, tag="rinv")
            nc.vector.reciprocal(rinv[:], sumexp[:])
            rinv_bf = constp.tile([H, 1], BF, tag="rinv_bf")
            with nc.allow_low_precision(reason="tiny scalar copy"):
                nc.vector.tensor_copy(rinv_bf[:], rinv[:])
            rb_ps = ps.tile([128, NT], F32, tag="ps_awb", name="rb_ps")
            for m in range(NT):
                nc.tensor.matmul(
                    rb_ps[:, m : m + 1],
                    exp_sb[:, m * 128 : (m + 1) * 128],
                    rinv_bf[:],
                    start=True,
                    stop=True,
                )
            rinvb_sb = constp.tile([128, NT], F32, tag="rinvb")
            nc.vector.tensor_copy(rinvb_sb[:], rb_ps[:])
            # normalized attention weights output (off the critical path)
            nc.vector.tensor_scalar_mul(sc_sb[:], sc_sb[:], rinv[:])
            nc.sync.dma_start(aw[:], sc_sb[:].bitcast(F32))

            # ---- fused V-projection + gate + O-projection, per s-chunk ----
            for n in range(NCH):
                cs = slice(n * 512, (n + 1) * 512)
                if n == 0:
                    vts = vt0
                else:
                    vts = []
                    for t in range(NT):
                        vt = vstr.tile([128, 512], BF, tag="vt", name=f"vt{n}_{t}")
                        nc.sync.dma_start(vt[:], valueT[t * 128 : (t + 1) * 128, cs])
                        vts.append(vt)
                attns = []
                for m in range(NT):
                    # awB[p, s] = aw[2m + p//64, s] via 0/1 expansion matmul
                    ab_ps = ps.tile([128, 512], F32, tag="ps_awb", name=f"ab{n}_{m}")
                    nc.tensor.matmul(
                        ab_ps[:],
                        exp_sb[:, m * 128 : (m + 1) * 128],
                        aw_bf[:, cs],
                        start=True,
                        stop=True,
                    )
                    awb_sb = awbp.tile([128, 512], F32, tag="awb", name=f"awb{n}_{m}")
                    nc.vector.tensor_scalar_mul(
                        awb_sb[:], ab_ps[:], rinvb_sb[:, m : m + 1]
                    )
                    # Vp^T tile: accumulate over e_in
                    v_ps = ps.tile([128, 512], F32, tag="ps_v", name=f"vps{n}_{m}")
                    for k in range(NT):
                        nc.tensor.matmul(
                            v_ps[:],
                            wv_sb[:, k * E + m * 128 : k * E + (m + 1) * 128],
                            vts[k][:],
                            start=(k == 0),
                            stop=(k == NT - 1),
                        )
                    # attn^T = (Vp^T + bv) * awB, single DVE op
                    attn_m = attnp.tile([128, 512], BF, tag="attn", name=f"at{n}_{m}")
                    nc.vector.scalar_tensor_tensor(
                        attn_m[:],
                        v_ps[:],
                        bvc_sb[:, m : m + 1],
                        awb_sb[:],
                        op0=AluOp.add,
                        op1=AluOp.mult,
                    )
                    attns.append(attn_m)
                for j in range(4):
                    for oc in range(2):
                        o_ps = ps.tile([128, 512], F32, tag="ps_o", name=f"o{n}_{j}_{oc}")
                        for m in range(NT):
                            nc.tensor.matmul(
                                o_ps[:],
                                attns[m][:, j * 128 : (j + 1) * 128],
                                wo_sb[:, m * E + oc * 512 : m * E + (oc + 1) * 512],
                                start=(m == 0),
                                stop=(m == NT - 1),
                            )
                        o_sb = outp.tile([128, 512], F32, tag="o", name=f"ot{n}_{j}_{oc}")
                        nc.vector.tensor_add(
                            o_sb[:], o_ps[:], bob_sb[:, oc * 512 : (oc + 1) * 512]
                        )
                        nc.sync.dma_start(
                            out[n * 512 + j * 128 : n * 512 + (j + 1) * 128,
                                oc * 512 : (oc + 1) * 512],
                            o_sb[:],
                        )

    nc.compile()
    return nc


def _get_compiled():
    global _compiled_nc
    if _compiled_nc is None:
        _compiled_nc = _build()
    return _compiled_nc


def kernel(**inputs):
    global LAST_RESULT
    nc = _get_compiled()

    query = np.asarray(inputs["query"], dtype=np.float32)
    key = np.asarray(inputs["key"], dtype=np.float32)
    value = np.asarray(inputs["value"], dtype=np.float32)
    mask = np.asarray(inputs["key_padding_mask"])
    Wq = np.ascontiguousarray(np.asarray(inputs["Wq"], dtype=np.float32))
    Wk = np.ascontiguousarray(np.asarray(inputs["Wk"], dtype=np.float32))
    Wv = np.asarray(inputs["Wv"], dtype=np.float32).astype(BF_NP)
    Wo = np.asarray(inputs["Wo"], dtype=np.float32).astype(BF_NP)
    bq = np.asarray(inputs["bq"], dtype=np.float32).reshape(1, E)
    bv = np.asarray(inputs["bv"], dtype=np.float32)
    bo = np.asarray(inputs["bo"], dtype=np.float32).reshape(1, E)

    expc = np.zeros((H, E), dtype=BF_NP)
    for h in range(H):
        expc[h, h * D : (h + 1) * D] = 1.0

    shared = {
        "Wq": Wq,
        "Wk": Wk,
        "Wv": Wv,
        "Wo": Wo,
        "bq": bq,
        "bv_col": np.ascontiguousarray(bv.reshape(NT, 128).T),
        "bo": bo,
        "ones": np.ones((1, E), dtype=np.float32),
        "EXP": expc,
    }
    in_maps = []
    for b in range(B):
        m = dict(shared)
        m["q_col"] = np.ascontiguousarray(query[b, 0].reshape(NT, 128).T)
        m["keyT"] = np.ascontiguousarray(key[b].T)
        m["valueT"] = np.ascontiguousarray(value[b].T.astype(BF_NP))
        m["maskbias"] = (
            ((mask[b] != 0).astype(np.float32) - 1.0) * 1e9
        ).reshape(1, S)
        in_maps.append(m)

    res = run_bass_kernel_spmd(nc, in_maps, list(range(N_CORES)))
    LAST_RESULT = res

    out = np.stack([res.results[b]["out"] for b in range(B)])
    aw3 = np.stack([res.results[b]["aw"] for b in range(B)])
    aw_exp = np.broadcast_to(aw3[..., None], (B, H, S, D))
    return out, aw_exp


# revision 13
# speedup vs baseline: 1.0795x; 1.0795x over previous
"""Trainium2 Bass kernel: batched single-query cross-attention (gated-V variant).

Reference computation (per batch b):
    Q  = query @ Wq + bq                     (1, E)
    K  = key @ Wk + bk                       (S, E)
    V  = value @ Wv + bv                     (S, E)
    scores[h, s] = <Q_head_h, K_head_h[s]> / sqrt(D)  (+ mask)
    aw = softmax_s(scores)                   (H, S)
    attn[s, e] = V[s, e] * aw[e // D, s]
    out = attn @ Wo + bo                     (S, E)
    returns (out, broadcast(aw to (B, H, S, D)))

Key algebraic facts exploited:
  * K is only consumed through scores, and scores = key @ A with
    A[e, h] = (1/8) * sum_d Wk[e, h*64+d] * Q[h*64+d]  -- the full K
    projection (8.6 GFLOP/core) collapses into a 1024x16 matrix A built
    with cheap elementwise ops. bk shifts every score of a head by the
    same constant, which softmax cancels, so bk is dropped entirely.
  * Everything is kept in "embedding-on-partitions" layout (x^T tiles) so
    all big matmuls use naturally-laid-out weights; key/value are
    transposed on the host (pure layout prep, zero FLOPs).

Sharding: data-parallel over the batch dim, one batch element per core
(B == 8 == n_cores). Weights are replicated.

Precision: the score/softmax chain runs in float32r (TF32-like PE mode,
~1.5e-4 matmul error) so the attention-weight output stays accurate; the
two heavy projections (V and output, 2 x 8.6 GFLOP/core) run in bf16,
which streams at twice the fp32r rate on the PE, accumulating in fp32.

Scheduling notes: a burst of K=1 filler matmuls is woven into the
DMA-bound warmup so the PE's HAM clock-gate reaches 2.4 GHz before the
score matmuls, and stays there through the keyT-streaming phase (idle
gaps stay under the ~3.4 us re-throttle window).
"""

import sys

import numpy as np

if "/opt/trn_rl_repo" not in sys.path:
    sys.path.insert(0, "/opt/trn_rl_repo")

import ml_dtypes  # noqa: E402

import concourse.bass as bass  # noqa: E402  # noqa: F401
import concourse.tile as tile  # noqa: E402
from concourse import bacc, mybir  # noqa: E402
from concourse.bass_utils import run_bass_kernel_spmd  # noqa: E402

B, S, E, H, D = 8, 4096, 1024, 16, 64
NT = E // 128  # 8 partition-tiles of the embedding dim
NCH = S // 512  # 8 sequence chunks
N_CORES = 8
F32 = mybir.dt.float32
FR = mybir.dt.float32r
BF = mybir.dt.bfloat16
BF_NP = ml_dtypes.bfloat16

LAST_RESULT = None
_compiled_nc = None


def _build():
    nc = bacc.Bacc(
        "TRN2", target_bir_lowering=False, debug=False, num_devices=N_CORES
    )

    def din(name, shape, dt=F32):
        return nc.dram_tensor(name, shape, dt, kind="ExternalInput").ap()

    def dout(name, shape):
        return nc.dram_tensor(name, shape, F32, kind="ExternalOutput").ap()

    # score-path tensors are float32r end to end (the BIR verifier requires
    # fp32r matmul operands to be produced as fp32r); V/O-path tensors bf16.
    q_col = din("q_col", (128, NT), BF)  # query[b] as column tiles: [p, k] = q[k*128+p]
    keyT = din("keyT", (E, S), FR)       # key[b].T
    valueT = din("valueT", (E, S), BF)   # value[b].T
    mb = din("maskbias", (1, S), FR)     # (mask-1)*1e9 : 0 keep, -1e9 masked
    Wq = din("Wq", (E, E), BF)
    Wk = din("Wk", (E, E), BF)           # only feeds the DVE (A build)
    Wv = din("Wv", (E, E), BF)
    Wo = din("Wo", (E, E), BF)
    bq = din("bq", (1, E))
    bv_col = din("bv_col", (128, NT))    # bv as column tiles
    bo = din("bo", (1, E))
    ones = din("ones", (1, E), FR)
    expm = din("EXP", (H, E), BF)
    out = dout("out", (S, E))
    aw = dout("aw", (H, S))

    AluOp = mybir.AluOpType
    Act = mybir.ActivationFunctionType

    with tile.TileContext(nc) as tc:
        with (
            tc.tile_pool(name="constp", bufs=1) as constp,
            tc.tile_pool(name="wres", bufs=1) as wres,
            tc.tile_pool(name="wstr", bufs=2) as wstr,
            tc.tile_pool(name="kstr", bufs=10) as kstr,
            tc.tile_pool(name="vstr", bufs=16) as vstr,
            tc.tile_pool(name="smax", bufs=1) as smax,
            tc.tile_pool(name="awbp", bufs=3) as awbp,
            tc.tile_pool(name="attnp", bufs=12) as attnp,
            tc.tile_pool(name="outp", bufs=4) as outp,
            tc.tile_pool(name="ps", bufs=2, space="PSUM") as ps,
        ):
            # ---- small constants; exp first so PE warmup fillers can start ----
            exp_sb = constp.tile([H, E], BF, tag="exp")
            nc.sync.dma_start(exp_sb[:], expm[:])
            q_sb = constp.tile([128, NT], BF, tag="q_col")
            nc.sync.dma_start(q_sb[:], q_col[:])
            bq_sb = constp.tile([1, E], F32, tag="bq")
            nc.sync.dma_start(bq_sb[:], bq[:])
            bvc_sb = constp.tile([128, NT], F32, tag="bvc")
            nc.sync.dma_start(bvc_sb[:], bv_col[:])
            bo_sb = constp.tile([1, E], F32, tag="bo")
            nc.sync.dma_start(bo_sb[:], bo[:])
            ones_sb = constp.tile([1, E], FR, tag="ones")
            nc.sync.dma_start(ones_sb[:], ones[:])
            mb_sb = constp.tile([1, S], FR, tag="mb")
            nc.sync.dma_start(mb_sb[:], mb[:])

            # HAM warmup: full-array (K=128) dead matmuls on a memset tile
            # keep the PE's activity monitor fed through the DMA-bound
            # startup so it upclocks to 2.4 GHz early.
            warm_sb = constp.tile([128, 512], BF, tag="warm")
            nc.vector.memset(warm_sb[:], 0.0)
            filler_n = iter(range(1000))

            def filler(count, rhs=None):
                for _ in range(count):
                    f_ps = ps.tile(
                        [128, 512], F32, tag="ps_s", name=f"fill{next(filler_n)}"
                    )
                    lhs = warm_sb[:, :128] if rhs is None else rhs[:, :128]
                    nc.tensor.matmul(
                        f_ps[:],
                        lhs,
                        warm_sb[:] if rhs is None else rhs[:],
                        start=True,
                        stop=True,
                    )

            filler(14)

            # ---- Q projection: Q = q @ Wq + bq, as a [1, E] row ----
            qrow_ps = [ps.tile([1, 512], F32, tag="ps_v", name=f"qrow{c}") for c in range(2)]
            for k in range(NT):
                wq_k = wstr.tile([128, E], BF, tag="w", name=f"wq{k}")
                nc.sync.dma_start(wq_k[:], Wq[k * 128 : (k + 1) * 128, :])
                for c in range(2):
                    nc.tensor.matmul(
                        qrow_ps[c][:],
                        q_sb[:, k : k + 1],
                        wq_k[:, c * 512 : (c + 1) * 512],
                        start=(k == 0),
                        stop=(k == NT - 1),
                    )
                filler(2, rhs=wq_k[:, :512])
            q_row = constp.tile([1, E], FR, tag="q_row")
            for c in range(2):
                nc.vector.tensor_add(
                    q_row[:, c * 512 : (c + 1) * 512],
                    qrow_ps[c][:],
                    bq_sb[:, c * 512 : (c + 1) * 512],
                )
            # fold in the 1/sqrt(D) score scale here (scores are linear in Q)
            nc.scalar.mul(q_row[:], q_row[:], 1.0 / 8.0)

            # ---- broadcast Q across partitions (PE rank-1 trick) ----
            qb_sb = constp.tile([128, E], F32, tag="qb")
            for c in range(2):
                qb_ps = ps.tile([128, 512], F32, tag="ps_v", name=f"qb{c}")
                nc.tensor.matmul(
                    qb_ps[:],
                    ones_sb[:1, :128],
                    q_row[:1, c * 512 : (c + 1) * 512],
                    start=True,
                    stop=True,
                )
                nc.vector.tensor_copy(qb_sb[:, c * 512 : (c + 1) * 512], qb_ps[:])

            # ---- A[e, h] = sum over the 64-wide head block of Wk * Qb;
            # keyT chunk-0 tiles prefetch behind each Wk tile ----
            a_sb = constp.tile([128, 128], FR, tag="A")
            kt0 = []
            for k in range(NT):
                wk_k = wstr.tile([128, E], BF, tag="wk", name=f"wk{k}", bufs=2)
                nc.sync.dma_start(wk_k[:], Wk[k * 128 : (k + 1) * 128, :])
                kt = kstr.tile([128, 512], FR, tag="kt", name=f"kt0_{k}")
                nc.sync.dma_start(kt[:], keyT[k * 128 : (k + 1) * 128, 0:512])
                kt0.append(kt)
                tmp = wstr.tile([128, E], F32, tag="w", name=f"tmpA{k}")
                nc.vector.tensor_mul(tmp[:], wk_k[:], qb_sb[:])
                with nc.allow_low_precision(reason="fp32r rounding of A"):
                    nc.vector.reduce_sum(
                        a_sb[:, k * H : (k + 1) * H],
                        tmp[:].rearrange("p (h d) -> p h d", d=D),
                        axis=mybir.AxisListType.X,
                    )
                filler(2, rhs=wk_k[:, :512])

            # ---- scores^T[h, s] = sum_e A[e, h] * keyT[e, s]  (+ mask) ----
            sc_sb = smax.tile([H, S], FR, tag="sc")
            aw_bf = smax.tile([H, S], BF, tag="awbf")
            shift_sb = constp.tile([H, 1], F32, tag="shift")
            nc.vector.memset(shift_sb[:], -12.0)
            for n in range(NCH):
                cs = slice(n * 512, (n + 1) * 512)
                s_ps = ps.tile([H, 512], F32, tag="ps_s", name=f"sps{n}")
                for t in range(NT):
                    if n == 0:
                        kt = kt0[t]
                    else:
                        kt = kstr.tile([128, 512], FR, tag="kt", name=f"kt{n}_{t}")
                        nc.sync.dma_start(kt[:], keyT[t * 128 : (t + 1) * 128, cs])
                    nc.tensor.matmul(
                        s_ps[:],
                        a_sb[:, t * H : (t + 1) * H],
                        kt[:],
                        start=(t == 0),
                        stop=False,
                    )
                nc.tensor.matmul(
                    s_ps[:],
                    ones_sb[:1, :H],
                    mb_sb[:1, cs],
                    start=False,
                    stop=True,
                )
                # online softmax: exp with a fixed safe shift (scores are
                # O(1); true max <= ~8 << 12), normalization folded in later
                nc.scalar.activation(sc_sb[:, cs], s_ps[:], Act.Exp, bias=shift_sb[:])
                with nc.allow_low_precision(reason="bf16 gate weights"):
                    nc.vector.tensor_copy(aw_bf[:, cs], sc_sb[:, cs])
                ktb = (kt0[0] if n == 0 else kt)[:].bitcast(BF)
                filler(3, rhs=ktb[:, :512])

            # ---- V/O weights + first value chunk; queued after the
            # latency-critical score-path loads, before they are needed ----
            wv_sb = wres.tile([128, NT * E], BF, tag="wv")
            wo_sb = wres.tile([128, NT * E], BF, tag="wo")
            for k in range(NT):
                nc.sync.dma_start(
                    wv_sb[:, k * E : (k + 1) * E], Wv[k * 128 : (k + 1) * 128, :]
                )
            vt0 = []
            for t in range(NT):
                vt = vstr.tile([128, 512], BF, tag="vt", name=f"vt0_{t}")
                nc.sync.dma_start(vt[:], valueT[t * 128 : (t + 1) * 128, 0:512])
                vt0.append(vt)
            for k in range(NT):
                nc.sync.dma_start(
                    wo_sb[:, k * E : (k + 1) * E], Wo[k * 128 : (k + 1) * 128, :]
                )
            # bo broadcast across partitions, exact, on the otherwise-idle GpSimd
            bob_sb = constp.tile([128, E], F32, tag="bob")
            nc.gpsimd.partition_broadcast(bob_sb[:], bo_sb[:1, :])

            # ---- softmax tail: global sum, reciprocal, and the per-row
            # expansion of 1/sum to the awB partition layout ----
            sumexp = constp.tile([H, 1], F32, tag="sumexp")
            nc.vector.reduce_sum(sumexp[:], sc_sb[:], axis=mybir.AxisListType.X)
            rinv = constp.tile([H, 1], F32, tag="rinv")
            nc.vector.reciprocal(rinv[:], sumexp[:])
            # normalized attention weights output (off the critical path)
            nc.vector.tensor_scalar_mul(sc_sb[:], sc_sb[:], rinv[:])
            nc.sync.dma_start(aw[:], sc_sb[:].bitcast(F32))

            # ---- fused V-projection + gate + O-projection, per s-chunk ----
            for n in range(NCH):
                cs = slice(n * 512, (n + 1) * 512)
                with nc.allow_low_precision(reason="bf16 gate weights"):
                    nc.vector.tensor_scalar_mul(aw_bf[:, cs], aw_bf[:, cs], rinv[:])
                if n == 0:
                    vts = vt0
                else:
                    vts = []
                    for t in range(NT):
                        vt = vstr.tile([128, 512], BF, tag="vt", name=f"vt{n}_{t}")
                        nc.sync.dma_start(vt[:], valueT[t * 128 : (t + 1) * 128, cs])
                        vts.append(vt)
                attns = []
                for m in range(NT):
                    # awB[p, s] = aw[2m + p//64, s] via 0/1 expansion matmul
                    ab_ps = ps.tile([128, 512], F32, tag="ps_awb", name=f"ab{n}_{m}")
                    nc.tensor.matmul(
                        ab_ps[:],
                        exp_sb[:, m * 128 : (m + 1) * 128],
                        aw_bf[:, cs],
                        start=True,
                        stop=True,
                    )
                    awb_sb = awbp.tile([128, 512], F32, tag="awb", name=f"awb{n}_{m}")
                    nc.vector.tensor_copy(awb_sb[:], ab_ps[:])
                    # Vp^T tile: accumulate over e_in
                    v_ps = ps.tile([128, 512], F32, tag="ps_v", name=f"vps{n}_{m}")
                    for k in range(NT):
                        nc.tensor.matmul(
                            v_ps[:],
                            wv_sb[:, k * E + m * 128 : k * E + (m + 1) * 128],
                            vts[k][:],
                            start=(k == 0),
                            stop=(k == NT - 1),
                        )
                    # attn^T = (Vp^T + bv) * awB, single DVE op
                    attn_m = attnp.tile([128, 512], BF, tag="attn", name=f"at{n}_{m}")
                    nc.vector.scalar_tensor_tensor(
                        attn_m[:],
                        v_ps[:],
                        bvc_sb[:, m : m + 1],
                        awb_sb[:],
                        op0=AluOp.add,
                        op1=AluOp.mult,
                    )
                    attns.append(attn_m)
                for j in range(4):
                    for oc in range(2):
                        o_ps = ps.tile([128, 512], F32, tag="ps_o", name=f"o{n}_{j}_{oc}")
                        for m in range(NT):
                            nc.tensor.matmul(
                                o_ps[:],
                                attns[m][:, j * 128 : (j + 1) * 128],
                                wo_sb[:, m * E + oc * 512 : m * E + (oc + 1) * 512],
                                start=(m == 0),
                                stop=(m == NT - 1),
                            )
                        o_sb = outp.tile([128, 512], F32, tag="o", name=f"ot{n}_{j}_{oc}")
                        nc.vector.tensor_add(
                            o_sb[:], o_ps[:], bob_sb[:, oc * 512 : (oc + 1) * 512]
                        )
                        nc.sync.dma_start(
                            out[n * 512 + j * 128 : n * 512 + (j + 1) * 128,
                                oc * 512 : (oc + 1) * 512],
                            o_sb[:],
                        )

    nc.compile()
    return nc


def _get_compiled():
    global _compiled_nc
    if _compiled_nc is None:
        _compiled_nc = _build()
    return _compiled_nc


def kernel(**inputs):
    global LAST_RESULT
    nc = _get_compiled()

    query = np.asarray(inputs["query"], dtype=np.float32)
    key = np.asarray(inputs["key"], dtype=np.float32)
    value = np.asarray(inputs["value"], dtype=np.float32)
    mask = np.asarray(inputs["key_padding_mask"])
    Wq = np.asarray(inputs["Wq"], dtype=np.float32).astype(BF_NP)
    Wk = np.asarray(inputs["Wk"], dtype=np.float32).astype(BF_NP)
    Wv = np.asarray(inputs["Wv"], dtype=np.float32).astype(BF_NP)
    Wo = np.asarray(inputs["Wo"], dtype=np.float32).astype(BF_NP)
    bq = np.asarray(inputs["bq"], dtype=np.float32).reshape(1, E)
    bv = np.asarray(inputs["bv"], dtype=np.float32)
    bo = np.asarray(inputs["bo"], dtype=np.float32).reshape(1, E)

    expc = np.zeros((H, E), dtype=BF_NP)
    for h in range(H):
        expc[h, h * D : (h + 1) * D] = 1.0

    shared = {
        "Wq": Wq,
        "Wk": Wk,
        "Wv": Wv,
        "Wo": Wo,
        "bq": bq,
        "bv_col": np.ascontiguousarray(bv.reshape(NT, 128).T),
        "bo": bo,
        "ones": np.ones((1, E), dtype=np.float32),
        "EXP": expc,
    }
    in_maps = []
    for b in range(B):
        m = dict(shared)
        m["q_col"] = np.ascontiguousarray(query[b, 0].reshape(NT, 128).T.astype(BF_NP))
        m["keyT"] = np.ascontiguousarray(key[b].T)
        m["valueT"] = np.ascontiguousarray(value[b].T.astype(BF_NP))
        m["maskbias"] = (
            ((mask[b] != 0).astype(np.float32) - 1.0) * 1e9
        ).reshape(1, S)
        in_maps.append(m)

    res = run_bass_kernel_spmd(nc, in_maps, list(range(N_CORES)))
    LAST_RESULT = res

    out = np.stack([res.results[b]["out"] for b in range(B)])
    aw3 = np.stack([res.results[b]["aw"] for b in range(B)])
    aw_exp = np.broadcast_to(aw3[..., None], (B, H, S, D))
    return out, aw_exp
